# revision 29
# baseline (speedup 1.0000x reference)
"""Trainium2 Bass kernel v6 for a 2-layer GCN with data-aware attention gate.

Math (per reference):
    src,dst = edges + self-loops; deg = bincount(dst); dinv = rsqrt(deg)
    norm = dinv[src]*dinv[dst]
    h1 = relu(segsum(norm * (x@W1)[src], dst) + b1)
    h2 = relu(segsum(norm * (h1@W2)[src], dst) + b2)
    out = h2 * sigmoid(h2@attn_w + attn_b)

v6 design (8 NeuronCores, dst-sharded, 128-slot windows):
  - Layer 1 exploits linearity: segsum(norm*(xW1)[src]) =
    (segsum(dinv_src*x[src]) * dinv_dst) @ W1.  The gather table is a
    HOST-prescaled (dinv*x) node-major bf16 tensor in table-row order --
    an ExternalInput, so layer 1 needs no device-side table build, no
    collective, and no copies.  Elements are single nodes (256B), indices
    are bucket-relative rows (int16-safe), no parity split.  W1 is
    applied per window right after aggregation (one extra matmul).
  - Layer 2 tables (dinv*(h1@W2), 64B rows zero-padded to 128B) are
    pair-packed 256B elements with a parity split, distributed in 3
    window-chunk buckets via AllGather+copy fired as soon as each
    chunk's flushes complete, overlapping layer-1 aggregation.
  - Near-exact gather streams per (group[, parity], bucket): per-window
    segments sized max-over-cores at element granularity (SPMD); chunks
    straddling window boundaries get one extra one-hot column per extra
    window (other windows' edges masked to -1).
  - Pieces are issued diagonally (A at slot g; C,B at slot g+1, issued
    before A of g+1) with matmuls for group g at slot g+2, so table
    buckets pipeline behind gather descriptor generation and the live
    gather-buffer window stays below NTAGS*GPBUFS.
  - Gathers round-robin the 4 SWDGE queues (each queue's descriptor
    generation runs on its own Q7 core pair).
  - Transposed aggregation: matmul(lhsT=gathered-chunk[128e x used],
    rhs=S[128e x 128 slots]) accumulates feat-major [used, 128] PSUM per
    window; self-loops via one identity matmul per window.
"""

import sys

import numpy as np

_CONC = "/opt/trn_rl_repo"
if _CONC not in sys.path:
    sys.path.insert(0, _CONC)

# ---------------------------------------------------------------------------
# configuration
# ---------------------------------------------------------------------------


class Cfg:
    def __init__(self, N=50000, DIN=128, DH=64, DOUT=32, NC=8, WPC=49, WPG=5,
                 GSPLIT=1024, NTAGS=12, GPBUFS=2, SHIFTB=1, SHIFTC=2):
        self.N, self.DIN, self.DH, self.DOUT = N, DIN, DH, DOUT
        self.NC, self.WPC, self.WPG = NC, WPC, WPG
        self.GSPLIT, self.NTAGS, self.GPBUFS = GSPLIT, NTAGS, GPBUFS
        self.SHIFTB, self.SHIFTC = SHIFTB, SHIFTC
        assert GSPLIT % 128 == 0
        self.groups = []
        w0 = 0
        while w0 < WPC:
            nw = min(WPG, WPC - w0)
            self.groups.append((w0, nw))
            w0 += nw
        self.G = len(self.groups)
        self.NPC = WPC * 128            # slots per core
        self.TOT = NC * self.NPC
        assert self.N <= self.TOT
        # table distribution buckets (window ranges; even slot counts)
        self.NB = 3
        self.tchunks = []               # (w_lo, w_hi, slots, table_base)
        base = 0
        for (w_lo, w_hi) in [(0, 20), (20, 40), (40, WPC)]:
            slots = (w_hi - w_lo) * 128
            self.tchunks.append((w_lo, w_hi, slots, base))
            base += NC * slots
        assert base == self.TOT


FULL = Cfg()

# ---------------------------------------------------------------------------
# host-side graph prep (structure only)
# ---------------------------------------------------------------------------


def _assign_slots(load, cfg):
    """LPT-deal nodes into NC*WPC bins of <=128 slots, balancing `load`."""
    import heapq

    nbins = cfg.NC * cfg.WPC
    order = np.argsort(-load, kind="stable")
    heap = [(0, b) for b in range(nbins)]
    heapq.heapify(heap)
    count = np.zeros(nbins, np.int64)
    pos = np.empty(cfg.N, np.int64)
    for n in order:
        l, b = heapq.heappop(heap)
        pos[n] = b * 128 + count[b]
        count[b] += 1
        if count[b] < 128:
            heapq.heappush(heap, (l + int(load[n]), b))
    return pos


def _build_streams(cfg, c_e, w_e, b_e, idx_e, h_e, dval_e, NH):
    """Shared stream/span/schedule builder for one layer.

    idx_e: bucket-relative gather element index per edge (int16-safe).
    h_e: parity half per edge (all zero when NH == 1).
    Returns (layerplan, ix_all, dvl_all).
    """
    NC, WPC, G, NB, GSPLIT = cfg.NC, cfg.WPC, cfg.G, cfg.NB, cfg.GSPLIT
    PC = GSPLIT // 128

    key = (((c_e * NH + h_e) * NB + b_e) * WPC + w_e) * 32768 + idx_e
    order_e = np.argsort(key, kind="stable")
    se_g = idx_e[order_e]
    se_d = dval_e[order_e]
    bkey = ((c_e * NH + h_e) * NB + b_e) * WPC + w_e
    se_key = bkey[order_e]
    bounds = np.searchsorted(se_key, np.arange(NC * NH * NB * WPC + 1))

    def bin_of(c, h, b, w):
        return ((c * NH + h) * NB + b) * WPC + w

    cnt = (bounds[1:] - bounds[:-1]).reshape(NC, NH, NB, WPC)
    segL = cnt.max(axis=0)                          # [NH, NB, WPC]

    elem_off = {}
    span_off = {}
    nspans = {}
    stream_L = {}
    seg_start = {}
    spans = {}
    pieces = []
    pieces_by_gb = {}
    wlist = [[[] for _ in range(nw)] for (w0, nw) in cfg.groups]
    etot = 0
    stot = 0
    for g, (w0, nw) in enumerate(cfg.groups):
        for h in range(NH):
            span_off[(g, h)] = stot
            for b in range(NB):
                starts = np.zeros(nw + 1, np.int64)
                for wl in range(nw):
                    starts[wl + 1] = starts[wl] + segL[h, b, w0 + wl]
                L = int(np.ceil(max(int(starts[nw]), 1) / 128) * 128)
                elem_off[(g, h, b)] = etot
                seg_start[(g, h, b)] = starts
                stream_L[(g, h, b)] = L
                plist = []
                o0 = 0
                while o0 < L:
                    nn = min(GSPLIT, L - o0)
                    pid = len(pieces)
                    pieces.append((g, h, b, etot + o0, nn))
                    plist.append(pid)
                    o0 += nn
                pieces_by_gb.setdefault((g, b), []).extend(plist)
                nch = L // 128
                sp_list = []
                for k2 in range(nch):
                    a2, b2 = k2 * 128, k2 * 128 + 128
                    wls = [wl for wl in range(nw)
                           if starts[wl] < b2 and starts[wl + 1] > a2]
                    if not wls:
                        wls = [nw - 1]
                    sp_list.append(wls)
                    for wl in wls:
                        pj = k2 // PC
                        off = k2 - pj * PC
                        wlist[g][wl].append((h, plist[pj], off, stot))
                        stot += 1
                spans[(g, h, b)] = sp_list
                etot += L
            nspans[(g, h)] = stot - span_off[(g, h)]
    idx_cols = etot // 16
    span_cols = stot
    max_nsp = max(nspans.values())

    NSLOT = G + cfg.SHIFTC - 1
    slots_sched = []
    for s in range(NSLOT + 1):
        ids = []
        for b, shift in ((2, cfg.SHIFTB), (1, cfg.SHIFTB), (0, 0)):
            g = s - shift
            if 0 <= g < G:
                ids.extend(pieces_by_gb.get((g, b), []))
        slots_sched.append(ids)

    ix_all = np.zeros((NC, 128, idx_cols), np.int16)
    dvl_all = np.full((NC, 128, span_cols), -1.0, np.float32)
    for c in range(NC):
        gi_full = np.zeros(etot, np.int64)
        dv_full = np.full(etot, -1.0, np.float32)
        wv_full = np.full(etot, -1, np.int64)
        for g, (w0, nw) in enumerate(cfg.groups):
            for h in range(NH):
                for b in range(NB):
                    base = elem_off[(g, h, b)]
                    starts = seg_start[(g, h, b)]
                    for wl in range(nw):
                        k = bin_of(c, h, b, w0 + wl)
                        lo, hi = bounds[k], bounds[k + 1]
                        n = hi - lo
                        o = base + starts[wl]
                        gi_full[o:o + n] = se_g[lo:hi]
                        dv_full[o:o + n] = se_d[lo:hi]
                        wv_full[o:o + n] = wl
        ix_all[c] = np.tile(
            gi_full.reshape(idx_cols, 16).T.astype(np.int16), (8, 1))
        scol = 0
        for g, (w0, nw) in enumerate(cfg.groups):
            for h in range(NH):
                for b in range(NB):
                    base = elem_off[(g, h, b)]
                    for k2, wls in enumerate(spans[(g, h, b)]):
                        sl = slice(base + k2 * 128, base + k2 * 128 + 128)
                        for wl in wls:
                            dvl_all[c, :, scol] = np.where(
                                wv_full[sl] == wl, dv_full[sl], -1.0)
                            scol += 1
        assert scol == span_cols

    lp = dict(idx_cols=idx_cols, span_cols=span_cols, max_nsp=max_nsp,
              span_off=span_off, nspans=nspans, pieces=pieces,
              slots=slots_sched, wlist=wlist, etot=etot, nslot=NSLOT,
              NH=NH)
    return lp, ix_all, dvl_all


def prep(x, edge_index, cfg):
    N, NC, WPC = cfg.N, cfg.NC, cfg.WPC
    NPC, DIN = cfg.NPC, cfg.DIN

    src = edge_index[0].astype(np.int64)
    dst = edge_index[1].astype(np.int64)
    deg_in = np.bincount(dst, minlength=N).astype(np.int64)
    deg = (deg_in + 1).astype(np.float32)          # + self-loop
    dinv = (1.0 / np.sqrt(np.maximum(deg, 1e-12))).astype(np.float32)

    pos = _assign_slots(deg_in, cfg)

    dinv_slot = np.ones(cfg.TOT, np.float32)
    dinv_slot[pos] = dinv

    # slot (core-major: c*NPC + s) -> (bucket, bucket-relative row)
    trow_b = np.empty(cfg.TOT, np.int64)
    trow_r = np.empty(cfg.TOT, np.int64)
    s_all = np.arange(NPC)
    for bi, (w_lo, w_hi, slots, base) in enumerate(cfg.tchunks):
        sel = s_all[(s_all >= w_lo * 128) & (s_all < w_hi * 128)]
        for c in range(NC):
            trow_b[c * NPC + sel] = bi
            trow_r[c * NPC + sel] = c * slots + (sel - w_lo * 128)

    s_pos = pos[src]
    d_pos = pos[dst]
    b_e = trow_b[s_pos]
    r_e = trow_r[s_pos]
    c_e = d_pos // NPC
    w_e = (d_pos % NPC) // 128
    dval_e = (d_pos % 128 + 2).astype(np.float32)

    # layer 1: single-node 256B elements, bucket-relative row index
    assert r_e.max() < 32768
    lp1, ix1, dvl1 = _build_streams(
        cfg, c_e, w_e, b_e, r_e, np.zeros_like(r_e), dval_e, NH=1)
    # layer 2: pair-packed 256B elements, parity split
    lp2, ix2, dvl2 = _build_streams(
        cfg, c_e, w_e, b_e, r_e >> 1, r_e & 1, dval_e, NH=2)

    import ml_dtypes
    bf16 = ml_dtypes.bfloat16

    # host-prescaled gather table for layer 1: (dinv*x) node-major,
    # table-row order; plus the own-shard copy for self-loop matmuls.
    X_all = np.zeros((cfg.TOT, DIN), np.float32)
    X_all[pos] = np.asarray(x, np.float32)
    Xs = X_all * dinv_slot[:, None]
    xnm = np.zeros((cfg.TOT, DIN), bf16)
    for bi, (w_lo, w_hi, slots, base) in enumerate(cfg.tchunks):
        sel = (trow_b == bi)
        rows = np.zeros((NC * slots, DIN), bf16)
        rows[trow_r[sel]] = Xs[sel].astype(bf16)
        xnm[base:base + NC * slots] = rows
    xown = np.zeros((NC, 128, NPC), bf16)
    for c in range(NC):
        sl = Xs[c * NPC:(c + 1) * NPC].astype(bf16)   # [NPC, 128]
        xown[c] = sl.reshape(WPC, 128, DIN).transpose(1, 0, 2).reshape(
            128, NPC)

    dv128 = np.zeros((NC, 128, WPC), np.float32)
    dvrep = np.zeros((NC, 64, NPC), np.float32)
    for c in range(NC):
        ds = dinv_slot[c * NPC:(c + 1) * NPC]
        dv128[c] = ds.reshape(WPC, 128).T
        dvrep[c] = np.tile(ds, (64, 1))

    plan = dict(lp1=lp1, lp2=lp2)
    host = dict(xnm=xnm, xown=xown, dv128=dv128, dvrep=dvrep,
                ix1=ix1, dvl1=dvl1.astype(bf16),
                ix2=ix2, dvl2=dvl2.astype(bf16),
                pos=pos, trow_b=trow_b, trow_r=trow_r)
    return plan, host


# ---------------------------------------------------------------------------
# device kernel
# ---------------------------------------------------------------------------


def build(cfg, plan):
    import concourse.bass as bass  # noqa: F401
    import concourse.mybir as mybir
    import concourse.tile as tile
    from concourse import bacc

    NC, WPC, G, NB = cfg.NC, cfg.WPC, cfg.G, cfg.NB
    NPC = cfg.NPC
    DH, DOUT = cfg.DH, cfg.DOUT
    GSPLIT, NTAGS = cfg.GSPLIT, cfg.NTAGS
    f32 = mybir.dt.float32
    bf16 = mybir.dt.bfloat16
    AF = mybir.ActivationFunctionType
    SHIFTC = cfg.SHIFTC
    lp1, lp2 = plan["lp1"], plan["lp2"]

    nc = bacc.Bacc(
        "TRN2", target_bir_lowering=False, debug=False,
        num_devices=NC, num_swdge_queues=4,
    )

    xnm_d = nc.dram_tensor("xnm", [cfg.TOT, 128], bf16,
                           kind="ExternalInput")
    xo_d = nc.dram_tensor("xown", [128, NPC], bf16, kind="ExternalInput")
    w1_d = nc.dram_tensor("w1", [128, DH], bf16, kind="ExternalInput")
    w2_d = nc.dram_tensor("w2", [DH, DOUT], bf16, kind="ExternalInput")
    dv128_d = nc.dram_tensor("dv128", [128, WPC], f32, kind="ExternalInput")
    dvrep_d = nc.dram_tensor("dvrep", [64, NPC], f32, kind="ExternalInput")
    id_d = nc.dram_tensor("ident", [128, 128], bf16, kind="ExternalInput")
    b1_d = nc.dram_tensor("b1c", [DH, 1], f32, kind="ExternalInput")
    b2_d = nc.dram_tensor("b2c", [DOUT, 1], f32, kind="ExternalInput")
    idaw_d = nc.dram_tensor("idaw", [DOUT, DOUT + 1], bf16,
                            kind="ExternalInput")
    ab_d = nc.dram_tensor("abc", [128, 1], f32, kind="ExternalInput")
    gi_d = nc.dram_tensor("gi128", [128, 128], bf16, kind="ExternalInput")
    ix1_d = nc.dram_tensor("ix1", [128, lp1["idx_cols"]], mybir.dt.int16,
                           kind="ExternalInput")
    dvl1_d = nc.dram_tensor("dvl1", [128, lp1["span_cols"]], bf16,
                            kind="ExternalInput")
    ix2_d = nc.dram_tensor("ix2", [128, lp2["idx_cols"]], mybir.dt.int16,
                           kind="ExternalInput")
    dvl2_d = nc.dram_tensor("dvl2", [128, lp2["span_cols"]], bf16,
                            kind="ExternalInput")
    out_d = nc.dram_tensor("out_sh", [NPC, DOUT], f32, kind="ExternalOutput")

    rg = [list(range(NC))]
    qctr = [0]

    with tile.TileContext(nc) as tc:
        with tc.tile_pool(name="const", bufs=1) as cpool:
            def load(pool, dram, shape, dt=f32, eng=None):
                t = pool.tile(shape, dt, tag=dram.name, name=dram.name + "_s")
                (eng or nc.sync).dma_start(t[:], dram.ap())
                return t

            w1_s = load(cpool, w1_d, [128, DH], bf16)
            dv128_s = load(cpool, dv128_d, [128, WPC])
            id_s = load(cpool, id_d, [128, 128], bf16)
            gi_s = load(cpool, gi_d, [128, 128], bf16)
            w2_s = load(cpool, w2_d, [DH, DOUT], bf16)
            dvrep_s = load(cpool, dvrep_d, [64, NPC], eng=nc.scalar)
            b1_s = load(cpool, b1_d, [DH, 1], eng=nc.scalar)
            b2_s = load(cpool, b2_d, [DOUT, 1], eng=nc.scalar)
            idaw_s = load(cpool, idaw_d, [DOUT, DOUT + 1], bf16,
                          eng=nc.scalar)
            ab_s = load(cpool, ab_d, [128, 1], eng=nc.scalar)
            ix2_s = load(cpool, ix2_d, [128, lp2["idx_cols"]],
                         mybir.dt.int16, eng=nc.scalar)
            dvl2_s = load(cpool, dvl2_d, [128, lp2["span_cols"]], bf16,
                          eng=nc.scalar)

            t2sb = cpool.tile([128, WPC * 64], bf16, tag="t2sb", name="t2sb")
            h2at = cpool.tile([128, WPC * 33], f32, tag="h2at", name="h2at")
            nc.vector.memset(t2sb[:], 0.0)

            with tc.tile_pool(name="dram", bufs=1, space="DRAM") as dpool:
                shard_t = {}
                full_t = {}
                loc_t = {}
                for bi, (w_lo, w_hi, slots, base) in enumerate(cfg.tchunks):
                    shard_t[bi] = dpool.tile(
                        [slots, DH], bf16, tag=f"sh2_{bi}", name=f"sh2_{bi}")
                    full_t[bi] = dpool.tile(
                        [NC * slots, DH], bf16, tag=f"fu2_{bi}",
                        name=f"fu2_{bi}", addr_space="Shared")
                    if bi < NB - 1:
                        # buckets whose AllGather lands mid-L1 get a local
                        # copy (Shared-space gathers drain slower); the
                        # last bucket is read Shared-direct since its
                        # collective only completes at the layer boundary.
                        loc_t[bi] = dpool.tile(
                            [NC * slots, DH], bf16, tag=f"lo2_{bi}",
                            name=f"lo2_{bi}")

                def distribute(bi):
                    w_lo, w_hi, slots, base = cfg.tchunks[bi]
                    sh, fu = shard_t[bi], full_t[bi]
                    nc.sync.dma_start(
                        sh[:].rearrange("(w p) f -> p w f", p=128),
                        t2sb[:, w_lo * 64:w_hi * 64].rearrange(
                            "p (w f) -> p w f", f=64))
                    nc.gpsimd.collective_compute(
                        "AllGather", mybir.AluOpType.bypass,
                        replica_groups=rg, ins=[sh[:]], outs=[fu[:]])
                    if bi in loc_t:
                        nc.sync.dma_start(loc_t[bi][:], fu[:])

                # ---- shared aggregation loop
                def aggregate(lp, fvs, ix_s, dvl_s, used, sl_lhs, flush_fn,
                              inject):
                    span_off, nspans = lp["span_off"], lp["nspans"]
                    max_nsp = lp["max_nsp"]
                    with (
                        tc.tile_pool(name="gp", bufs=cfg.GPBUFS) as gp,
                        tc.tile_pool(name="sp", bufs=2) as sp,
                        tc.tile_pool(name="aps", bufs=4, space="PSUM") as aps,
                        tc.tile_pool(name="fsb", bufs=3) as fsb,
                        tc.tile_pool(name="fps", bufs=2, space="PSUM") as fps,
                    ):
                        Ss = {}
                        ptiles = {}
                        for s in range(lp["nslot"] + 1):
                            gS = s - SHIFTC + 1
                            if 0 <= gS < G:
                                for h in range(lp["NH"]):
                                    nsp = nspans[(gS, h)]
                                    sc0 = span_off[(gS, h)]
                                    S = sp.tile([128, max_nsp * 128], bf16,
                                                tag=f"S{h}", name=f"S{h}")
                                    nc.vector.tensor_tensor(
                                        out=S[:, :nsp * 128].rearrange(
                                            "p (c j) -> p c j", j=128),
                                        in0=dvl_s[:, sc0:sc0 + nsp]
                                        .unsqueeze(2)
                                        .broadcast_to((128, nsp, 128)),
                                        in1=gi_s[:].unsqueeze(1)
                                        .broadcast_to((128, nsp, 128)),
                                        op=mybir.AluOpType.is_equal,
                                    )
                                    Ss[(gS, h)] = S
                            gM = s - SHIFTC
                            if 0 <= gM < G:
                                w0, nw = cfg.groups[gM]
                                for wl in range(nw):
                                    w = w0 + wl
                                    mlist = lp["wlist"][gM][wl]
                                    ps = aps.tile([used, 128], f32,
                                                  tag="agg", name="agg")
                                    nc.tensor.matmul(
                                        ps[:], lhsT=sl_lhs(w), rhs=id_s[:],
                                        start=True, stop=(len(mlist) == 0))
                                    for i, (h, pid, off, scol) in enumerate(
                                            mlist):
                                        pt = ptiles[pid]
                                        bb = off * 128 + h * 64
                                        sc = (scol - span_off[(gM, h)]) * 128
                                        nc.tensor.matmul(
                                            ps[:],
                                            lhsT=pt[:, bb:bb + used],
                                            rhs=Ss[(gM, h)][:, sc:sc + 128],
                                            start=False,
                                            stop=(i == len(mlist) - 1))
                                    flush_fn(w, ps, fsb, fps)
                            for pid in lp["slots"][s]:
                                (g, h, b, eoff, nn) = lp["pieces"][pid]
                                pt = gp.tile([128, GSPLIT], bf16,
                                             tag=f"gt{qctr[0] % NTAGS}",
                                             name=f"gt{qctr[0] % NTAGS}")
                                nc.gpsimd.dma_gather(
                                    out_ap=pt[:, 0:nn].rearrange(
                                        "p (c d) -> p c d", d=128),
                                    in_ap=fvs[b],
                                    idxs_ap=ix_s[:, eoff // 16:
                                                 (eoff + nn) // 16],
                                    num_idxs=nn, num_idxs_reg=nn,
                                    elem_size=128, elem_step=128,
                                    queue_num=qctr[0] % 4,
                                    single_packet=False,
                                )
                                qctr[0] += 1
                                ptiles[pid] = pt
                            inject(s)

                # ---- layer-1 flush: ps [128 xfeat, 128 slots] ->
                # zT bf16 -> W1 projection (feat-major) -> dinv,relu ->
                # t2 table values
                def flush1(w, ps, fsb, fps):
                    zT = fsb.tile([128, 128], bf16, tag="zT", name="zT")
                    nc.scalar.activation(zT[:], ps[:], func=AF.Copy)
                    hps = fps.tile([DH, 128], f32, tag="hps", name="hps")
                    nc.tensor.matmul(hps[:], lhsT=w1_s[:], rhs=zT[:],
                                     start=True, stop=True)
                    a = fsb.tile([64, 128], f32, tag="a", name="a")
                    nc.vector.tensor_tensor(
                        out=a[:], in0=hps[:],
                        in1=dvrep_s[:, w * 128:(w + 1) * 128],
                        op=mybir.AluOpType.mult)
                    hT = fsb.tile([64, 128], bf16, tag="hT", name="hT")
                    nc.scalar.activation(hT[:], a[:], func=AF.Relu,
                                         bias=b1_s[:, 0:1])
                    t2ps = fps.tile([128, DOUT], f32, tag="t2ps", name="t2ps")
                    nc.tensor.matmul(t2ps[:], lhsT=hT[:], rhs=w2_s[:],
                                     start=True, stop=True)
                    nc.scalar.activation(
                        t2sb[:, w * 64:w * 64 + DOUT], t2ps[:],
                        func=AF.Copy, scale=dv128_s[:, w:w + 1])

                def sl1(w):
                    return xo_s[:, w * 128:(w + 1) * 128]

                # t2 bucket bi: fire its distribute two slots after its
                # last group's matmuls; the final bucket fires from L2's
                # slot 0 (its readers are C pieces at slot >= 1).
                fire_at = {}
                for bi, (w_lo, w_hi, slots, base) in enumerate(
                        cfg.tchunks[:-1]):
                    gl = max(g for g, (gw0, gnw) in enumerate(cfg.groups)
                             if gw0 < w_hi)
                    fire_at.setdefault(
                        min(gl + SHIFTC + 1, lp1["nslot"]), []).append(bi)

                def inject1(s):
                    for bi in fire_at.get(s, []):
                        distribute(bi)

                fvs1 = [xnm_d.ap()[base:base + NC * slots, :]
                        for (w_lo, w_hi, slots, base) in cfg.tchunks]
                with tc.tile_pool(name="pl1", bufs=1) as pl1:
                    xo_s = load(pl1, xo_d, [128, NPC], bf16)
                    ix1_s = load(pl1, ix1_d, [128, lp1["idx_cols"]],
                                 mybir.dt.int16)
                    dvl1_s = load(pl1, dvl1_d, [128, lp1["span_cols"]],
                                  bf16)
                    aggregate(lp1, fvs1, ix1_s, dvl1_s, 128, sl1, flush1,
                              inject1)

                # ---- layer-2 flush
                def flush2(w, ps, fsb, fps):
                    a2 = fsb.tile([DOUT, 128], f32, tag="a2", name="a2")
                    nc.vector.tensor_tensor(
                        out=a2[:], in0=ps[:],
                        in1=dvrep_s[:DOUT, w * 128:(w + 1) * 128],
                        op=mybir.AluOpType.mult)
                    h2T = fsb.tile([DOUT, 128], bf16, tag="h2T", name="h2T")
                    nc.scalar.activation(h2T[:], a2[:], func=AF.Relu,
                                         bias=b2_s[:, 0:1])
                    gps = fps.tile([128, DOUT + 1], f32, tag="gps", name="gps")
                    nc.tensor.matmul(gps[:], lhsT=h2T[:], rhs=idaw_s[:],
                                     start=True, stop=True)
                    nc.scalar.activation(
                        h2at[:, w * 33:(w + 1) * 33], gps[:], func=AF.Copy)

                def sl2(w):
                    return t2sb[:, w * 64:w * 64 + DOUT]

                def inject2(s):
                    if s == 0:
                        distribute(cfg.NB - 1)

                fvs2 = [(loc_t[bi] if bi in loc_t else full_t[bi])[:]
                        .rearrange("(a b) d -> a (b d)", b=2)
                        for bi in range(NB)]
                aggregate(lp2, fvs2, ix2_s, dvl2_s, DOUT, sl2, flush2,
                          inject2)

                # ---- attention gate tail
                with tc.tile_pool(name="tail", bufs=1) as tp:
                    atall = tp.tile([128, WPC], f32, tag="atall", name="atall")
                    nc.scalar.activation(
                        atall[:],
                        h2at[:].rearrange("p (w q) -> p w q", q=33)[:, :, 32],
                        func=AF.Sigmoid, bias=ab_s[:, 0:1])
                    oall = tp.tile([128, WPC * DOUT], f32, tag="oall",
                                   name="oall")
                    nc.vector.tensor_tensor(
                        out=oall[:].rearrange("p (w f) -> p w f", f=DOUT),
                        in0=h2at[:].rearrange(
                            "p (w q) -> p w q", q=33)[:, :, 0:DOUT],
                        in1=atall[:].unsqueeze(2)
                        .broadcast_to((128, WPC, DOUT)),
                        op=mybir.AluOpType.mult)
                    nc.sync.dma_start(
                        out_d.ap().rearrange("(w p) f -> p w f", p=128),
                        oall[:].rearrange("p (w f) -> p w f", f=DOUT))

    nc.compile()
    return nc


# ---------------------------------------------------------------------------
# entry point
# ---------------------------------------------------------------------------


def _make_in_maps(cfg, host, W1, b1, W2, b2, attn_w, attn_b):
    import ml_dtypes
    bf16 = ml_dtypes.bfloat16
    giota = np.tile(np.arange(2, 130, dtype=np.float32),
                    (128, 1)).astype(bf16)
    idaw = np.concatenate(
        [np.eye(cfg.DOUT, dtype=np.float32),
         np.asarray(attn_w, np.float32).reshape(cfg.DOUT, 1)],
        axis=1).astype(bf16)
    in_maps = []
    for c in range(cfg.NC):
        in_maps.append({
            "xnm": host["xnm"],
            "xown": host["xown"][c],
            "w1": np.asarray(W1, np.float32).astype(bf16),
            "w2": np.asarray(W2, np.float32).astype(bf16),
            "dv128": host["dv128"][c],
            "dvrep": host["dvrep"][c],
            "ident": np.eye(128, dtype=np.float32).astype(bf16),
            "b1c": np.asarray(b1, np.float32).reshape(cfg.DH, 1),
            "b2c": np.asarray(b2, np.float32).reshape(cfg.DOUT, 1),
            "idaw": idaw,
            "abc": np.full((128, 1),
                           np.asarray(attn_b, np.float32).reshape(-1)[0],
                           np.float32),
            "gi128": giota,
            "ix1": host["ix1"][c],
            "dvl1": host["dvl1"][c],
            "ix2": host["ix2"][c],
            "dvl2": host["dvl2"][c],
        })
    return in_maps


def run(x, edge_index, W1, b1, W2, b2, attn_w, attn_b, cfg=None,
        backend="hw", trace=False):
    cfg = cfg or FULL
    plan, host = prep(x, edge_index, cfg)
    nc = build(cfg, plan)
    in_maps = _make_in_maps(cfg, host, W1, b1, W2, b2, attn_w, attn_b)

    if backend == "sim":
        from concourse.bass_interp import MultiCoreSim
        sim = MultiCoreSim(nc, num_cores=cfg.NC, trace=False)
        for c, core in enumerate(sim.cores.values()):
            for name, arr in in_maps[c].items():
                core.tensor(name)[:] = arr
        sim.simulate()
        outs = [core.tensor("out_sh").copy() for core in sim.cores.values()]
        exec_ns = None
    else:
        from concourse import bass_utils
        from concourse.bass_interp import get_hw_module
        old = nc.m
        nc.m = get_hw_module(nc.m)
        try:
            res = bass_utils.run_bass_kernel_spmd(
                nc, in_maps, core_ids=list(range(cfg.NC)), trace=trace)
        finally:
            nc.m = old
        outs = [res.results[c]["out_sh"] for c in range(cfg.NC)]
        exec_ns = res.exec_time_ns

    full = np.concatenate(outs, axis=0)   # [TOT, DOUT] in slot order
    out = full[host["pos"]]               # unpermute -> [N, DOUT]
    return np.ascontiguousarray(out), exec_ns


def kernel(x, edge_index, W1, b1, W2, b2, attn_w, attn_b):
    out, _ = run(x, edge_index, W1, b1, W2, b2, attn_w, attn_b,
                 cfg=FULL, backend="hw", trace=False)
    return out


# revision 30
# speedup vs baseline: 1.0162x; 1.0162x over previous
"""Trainium2 Bass kernel v6 for a 2-layer GCN with data-aware attention gate.

Math (per reference):
    src,dst = edges + self-loops; deg = bincount(dst); dinv = rsqrt(deg)
    norm = dinv[src]*dinv[dst]
    h1 = relu(segsum(norm * (x@W1)[src], dst) + b1)
    h2 = relu(segsum(norm * (h1@W2)[src], dst) + b2)
    out = h2 * sigmoid(h2@attn_w + attn_b)

v6 design (8 NeuronCores, dst-sharded, 128-slot windows):
  - Layer 1 exploits linearity: segsum(norm*(xW1)[src]) =
    (segsum(dinv_src*x[src]) * dinv_dst) @ W1.  The gather table is a
    HOST-prescaled (dinv*x) node-major bf16 tensor in table-row order --
    an ExternalInput, so layer 1 needs no device-side table build, no
    collective, and no copies.  Elements are single nodes (256B), indices
    are bucket-relative rows (int16-safe), no parity split.  W1 is
    applied per window right after aggregation (one extra matmul).
  - Layer 2 tables (dinv*(h1@W2), 64B rows zero-padded to 128B) are
    pair-packed 256B elements with a parity split, distributed in 3
    window-chunk buckets via AllGather+copy fired as soon as each
    chunk's flushes complete, overlapping layer-1 aggregation.
  - Near-exact gather streams per (group[, parity], bucket): per-window
    segments sized max-over-cores at element granularity (SPMD); chunks
    straddling window boundaries get one extra one-hot column per extra
    window (other windows' edges masked to -1).
  - Pieces are issued diagonally (A at slot g; C,B at slot g+1, issued
    before A of g+1) with matmuls for group g at slot g+2, so table
    buckets pipeline behind gather descriptor generation and the live
    gather-buffer window stays below NTAGS*GPBUFS.
  - Gathers round-robin the 4 SWDGE queues (each queue's descriptor
    generation runs on its own Q7 core pair).
  - Transposed aggregation: matmul(lhsT=gathered-chunk[128e x used],
    rhs=S[128e x 128 slots]) accumulates feat-major [used, 128] PSUM per
    window; self-loops via one identity matmul per window.
"""

import sys

import numpy as np

_CONC = "/opt/trn_rl_repo"
if _CONC not in sys.path:
    sys.path.insert(0, _CONC)

# ---------------------------------------------------------------------------
# configuration
# ---------------------------------------------------------------------------


class Cfg:
    def __init__(self, N=50000, DIN=128, DH=64, DOUT=32, NC=8, WPC=49, WPG=5,
                 GSPLIT=1024, NTAGS=12, GPBUFS=2, SHIFTB=1, SHIFTC=2):
        self.N, self.DIN, self.DH, self.DOUT = N, DIN, DH, DOUT
        self.NC, self.WPC, self.WPG = NC, WPC, WPG
        self.GSPLIT, self.NTAGS, self.GPBUFS = GSPLIT, NTAGS, GPBUFS
        self.SHIFTB, self.SHIFTC = SHIFTB, SHIFTC
        assert GSPLIT % 128 == 0
        self.groups = []
        w0 = 0
        while w0 < WPC:
            nw = min(WPG, WPC - w0)
            self.groups.append((w0, nw))
            w0 += nw
        self.G = len(self.groups)
        self.NPC = WPC * 128            # slots per core
        self.TOT = NC * self.NPC
        assert self.N <= self.TOT
        # table distribution buckets (window ranges; even slot counts)
        self.NB = 3
        self.tchunks = []               # (w_lo, w_hi, slots, table_base)
        base = 0
        for (w_lo, w_hi) in [(0, 20), (20, 40), (40, WPC)]:
            slots = (w_hi - w_lo) * 128
            self.tchunks.append((w_lo, w_hi, slots, base))
            base += NC * slots
        assert base == self.TOT


FULL = Cfg()

# ---------------------------------------------------------------------------
# host-side graph prep (structure only)
# ---------------------------------------------------------------------------


def _assign_slots(load, cfg):
    """LPT-deal nodes into NC*WPC bins of <=128 slots, balancing `load`."""
    import heapq

    nbins = cfg.NC * cfg.WPC
    order = np.argsort(-load, kind="stable")
    heap = [(0, b) for b in range(nbins)]
    heapq.heapify(heap)
    count = np.zeros(nbins, np.int64)
    pos = np.empty(cfg.N, np.int64)
    for n in order:
        l, b = heapq.heappop(heap)
        pos[n] = b * 128 + count[b]
        count[b] += 1
        if count[b] < 128:
            heapq.heappush(heap, (l + int(load[n]), b))
    return pos


def _build_streams(cfg, c_e, w_e, b_e, idx_e, h_e, dval_e, NH):
    """Shared stream/span/schedule builder for one layer.

    idx_e: bucket-relative gather element index per edge (int16-safe).
    h_e: parity half per edge (all zero when NH == 1).
    Returns (layerplan, ix_all, dvl_all).
    """
    NC, WPC, G, NB, GSPLIT = cfg.NC, cfg.WPC, cfg.G, cfg.NB, cfg.GSPLIT
    PC = GSPLIT // 128

    key = (((c_e * NH + h_e) * NB + b_e) * WPC + w_e) * 32768 + idx_e
    order_e = np.argsort(key, kind="stable")
    se_g = idx_e[order_e]
    se_d = dval_e[order_e]
    bkey = ((c_e * NH + h_e) * NB + b_e) * WPC + w_e
    se_key = bkey[order_e]
    bounds = np.searchsorted(se_key, np.arange(NC * NH * NB * WPC + 1))

    def bin_of(c, h, b, w):
        return ((c * NH + h) * NB + b) * WPC + w

    cnt = (bounds[1:] - bounds[:-1]).reshape(NC, NH, NB, WPC)
    segL = cnt.max(axis=0)                          # [NH, NB, WPC]

    elem_off = {}
    span_off = {}
    nspans = {}
    stream_L = {}
    seg_start = {}
    spans = {}
    pieces = []
    pieces_by_gb = {}
    wlist = [[[] for _ in range(nw)] for (w0, nw) in cfg.groups]
    etot = 0
    stot = 0
    for g, (w0, nw) in enumerate(cfg.groups):
        for h in range(NH):
            span_off[(g, h)] = stot
            for b in range(NB):
                starts = np.zeros(nw + 1, np.int64)
                for wl in range(nw):
                    starts[wl + 1] = starts[wl] + segL[h, b, w0 + wl]
                L = int(np.ceil(max(int(starts[nw]), 1) / 128) * 128)
                elem_off[(g, h, b)] = etot
                seg_start[(g, h, b)] = starts
                stream_L[(g, h, b)] = L
                plist = []
                o0 = 0
                while o0 < L:
                    nn = min(GSPLIT, L - o0)
                    pid = len(pieces)
                    pieces.append((g, h, b, etot + o0, nn))
                    plist.append(pid)
                    o0 += nn
                pieces_by_gb.setdefault((g, b), []).extend(plist)
                nch = L // 128
                sp_list = []
                for k2 in range(nch):
                    a2, b2 = k2 * 128, k2 * 128 + 128
                    wls = [wl for wl in range(nw)
                           if starts[wl] < b2 and starts[wl + 1] > a2]
                    if not wls:
                        wls = [nw - 1]
                    sp_list.append(wls)
                    for wl in wls:
                        pj = k2 // PC
                        off = k2 - pj * PC
                        wlist[g][wl].append((h, plist[pj], off, stot))
                        stot += 1
                spans[(g, h, b)] = sp_list
                etot += L
            nspans[(g, h)] = stot - span_off[(g, h)]
    idx_cols = etot // 16
    span_cols = stot
    max_nsp = max(nspans.values())

    NSLOT = G + cfg.SHIFTC - 1
    slots_sched = []
    for s in range(NSLOT + 1):
        ids = []
        for b, shift in ((2, cfg.SHIFTB), (1, cfg.SHIFTB), (0, 0)):
            g = s - shift
            if 0 <= g < G:
                ids.extend(pieces_by_gb.get((g, b), []))
        slots_sched.append(ids)

    ix_all = np.zeros((NC, 128, idx_cols), np.int16)
    dvl_all = np.full((NC, 128, span_cols), -1.0, np.float32)
    for c in range(NC):
        gi_full = np.zeros(etot, np.int64)
        dv_full = np.full(etot, -1.0, np.float32)
        wv_full = np.full(etot, -1, np.int64)
        for g, (w0, nw) in enumerate(cfg.groups):
            for h in range(NH):
                for b in range(NB):
                    base = elem_off[(g, h, b)]
                    starts = seg_start[(g, h, b)]
                    for wl in range(nw):
                        k = bin_of(c, h, b, w0 + wl)
                        lo, hi = bounds[k], bounds[k + 1]
                        n = hi - lo
                        o = base + starts[wl]
                        gi_full[o:o + n] = se_g[lo:hi]
                        dv_full[o:o + n] = se_d[lo:hi]
                        wv_full[o:o + n] = wl
        ix_all[c] = np.tile(
            gi_full.reshape(idx_cols, 16).T.astype(np.int16), (8, 1))
        scol = 0
        for g, (w0, nw) in enumerate(cfg.groups):
            for h in range(NH):
                for b in range(NB):
                    base = elem_off[(g, h, b)]
                    for k2, wls in enumerate(spans[(g, h, b)]):
                        sl = slice(base + k2 * 128, base + k2 * 128 + 128)
                        for wl in wls:
                            dvl_all[c, :, scol] = np.where(
                                wv_full[sl] == wl, dv_full[sl], -1.0)
                            scol += 1
        assert scol == span_cols

    lp = dict(idx_cols=idx_cols, span_cols=span_cols, max_nsp=max_nsp,
              span_off=span_off, nspans=nspans, pieces=pieces,
              slots=slots_sched, wlist=wlist, etot=etot, nslot=NSLOT,
              NH=NH)
    return lp, ix_all, dvl_all


def prep(x, edge_index, cfg):
    N, NC, WPC = cfg.N, cfg.NC, cfg.WPC
    NPC, DIN = cfg.NPC, cfg.DIN

    src = edge_index[0].astype(np.int64)
    dst = edge_index[1].astype(np.int64)
    deg_in = np.bincount(dst, minlength=N).astype(np.int64)
    deg = (deg_in + 1).astype(np.float32)          # + self-loop
    dinv = (1.0 / np.sqrt(np.maximum(deg, 1e-12))).astype(np.float32)

    pos = _assign_slots(deg_in, cfg)

    dinv_slot = np.ones(cfg.TOT, np.float32)
    dinv_slot[pos] = dinv

    # slot (core-major: c*NPC + s) -> (bucket, bucket-relative row)
    trow_b = np.empty(cfg.TOT, np.int64)
    trow_r = np.empty(cfg.TOT, np.int64)
    s_all = np.arange(NPC)
    for bi, (w_lo, w_hi, slots, base) in enumerate(cfg.tchunks):
        sel = s_all[(s_all >= w_lo * 128) & (s_all < w_hi * 128)]
        for c in range(NC):
            trow_b[c * NPC + sel] = bi
            trow_r[c * NPC + sel] = c * slots + (sel - w_lo * 128)

    s_pos = pos[src]
    d_pos = pos[dst]
    b_e = trow_b[s_pos]
    r_e = trow_r[s_pos]
    c_e = d_pos // NPC
    w_e = (d_pos % NPC) // 128
    dval_e = (d_pos % 128 + 2).astype(np.float32)

    # layer 1: single-node 256B elements, bucket-relative row index
    assert r_e.max() < 32768
    lp1, ix1, dvl1 = _build_streams(
        cfg, c_e, w_e, b_e, r_e, np.zeros_like(r_e), dval_e, NH=1)
    # layer 2: pair-packed 256B elements, parity split
    lp2, ix2, dvl2 = _build_streams(
        cfg, c_e, w_e, b_e, r_e >> 1, r_e & 1, dval_e, NH=2)

    import ml_dtypes
    bf16 = ml_dtypes.bfloat16

    # host-prescaled gather table for layer 1: (dinv*x) node-major,
    # table-row order; plus the own-shard copy for self-loop matmuls.
    X_all = np.zeros((cfg.TOT, DIN), np.float32)
    X_all[pos] = np.asarray(x, np.float32)
    Xs = X_all * dinv_slot[:, None]
    xnm = np.zeros((cfg.TOT, DIN), bf16)
    for bi, (w_lo, w_hi, slots, base) in enumerate(cfg.tchunks):
        sel = (trow_b == bi)
        rows = np.zeros((NC * slots, DIN), bf16)
        rows[trow_r[sel]] = Xs[sel].astype(bf16)
        xnm[base:base + NC * slots] = rows
    xown = np.zeros((NC, 128, NPC), bf16)
    for c in range(NC):
        sl = Xs[c * NPC:(c + 1) * NPC].astype(bf16)   # [NPC, 128]
        xown[c] = sl.reshape(WPC, 128, DIN).transpose(1, 0, 2).reshape(
            128, NPC)

    dv128 = np.zeros((NC, 128, WPC), np.float32)
    dvrep = np.zeros((NC, 64, NPC), np.float32)
    for c in range(NC):
        ds = dinv_slot[c * NPC:(c + 1) * NPC]
        dv128[c] = ds.reshape(WPC, 128).T
        dvrep[c] = np.tile(ds, (64, 1))

    plan = dict(lp1=lp1, lp2=lp2)
    host = dict(xnm=xnm, xown=xown, dv128=dv128, dvrep=dvrep,
                ix1=ix1, dvl1=dvl1.astype(bf16),
                ix2=ix2, dvl2=dvl2.astype(bf16),
                pos=pos, trow_b=trow_b, trow_r=trow_r)
    return plan, host


# ---------------------------------------------------------------------------
# device kernel
# ---------------------------------------------------------------------------


def build(cfg, plan):
    import concourse.bass as bass  # noqa: F401
    import concourse.mybir as mybir
    import concourse.tile as tile
    from concourse import bacc

    NC, WPC, G, NB = cfg.NC, cfg.WPC, cfg.G, cfg.NB
    NPC = cfg.NPC
    DH, DOUT = cfg.DH, cfg.DOUT
    GSPLIT, NTAGS = cfg.GSPLIT, cfg.NTAGS
    f32 = mybir.dt.float32
    bf16 = mybir.dt.bfloat16
    AF = mybir.ActivationFunctionType
    SHIFTC = cfg.SHIFTC
    lp1, lp2 = plan["lp1"], plan["lp2"]

    nc = bacc.Bacc(
        "TRN2", target_bir_lowering=False, debug=False,
        num_devices=NC, num_swdge_queues=4,
    )

    xnm_d = nc.dram_tensor("xnm", [cfg.TOT, 128], bf16,
                           kind="ExternalInput")
    xo_d = nc.dram_tensor("xown", [128, NPC], bf16, kind="ExternalInput")
    w1_d = nc.dram_tensor("w1", [128, DH], bf16, kind="ExternalInput")
    w2_d = nc.dram_tensor("w2", [DH, DOUT], bf16, kind="ExternalInput")
    dv128_d = nc.dram_tensor("dv128", [128, WPC], f32, kind="ExternalInput")
    dvrep_d = nc.dram_tensor("dvrep", [64, NPC], f32, kind="ExternalInput")
    id_d = nc.dram_tensor("ident", [128, 128], bf16, kind="ExternalInput")
    b1_d = nc.dram_tensor("b1c", [DH, 1], f32, kind="ExternalInput")
    b2_d = nc.dram_tensor("b2c", [DOUT, 1], f32, kind="ExternalInput")
    idaw_d = nc.dram_tensor("idaw", [DOUT, DOUT + 1], bf16,
                            kind="ExternalInput")
    ab_d = nc.dram_tensor("abc", [128, 1], f32, kind="ExternalInput")
    gi_d = nc.dram_tensor("gi128", [128, 128], bf16, kind="ExternalInput")
    ix1_d = nc.dram_tensor("ix1", [128, lp1["idx_cols"]], mybir.dt.int16,
                           kind="ExternalInput")
    dvl1_d = nc.dram_tensor("dvl1", [128, lp1["span_cols"]], bf16,
                            kind="ExternalInput")
    ix2_d = nc.dram_tensor("ix2", [128, lp2["idx_cols"]], mybir.dt.int16,
                           kind="ExternalInput")
    dvl2_d = nc.dram_tensor("dvl2", [128, lp2["span_cols"]], bf16,
                            kind="ExternalInput")
    out_d = nc.dram_tensor("out_sh", [NPC, DOUT], f32, kind="ExternalOutput")

    rg = [list(range(NC))]
    qctr = [0]

    with tile.TileContext(nc) as tc:
        with tc.tile_pool(name="const", bufs=1) as cpool:
            def load(pool, dram, shape, dt=f32, eng=None):
                t = pool.tile(shape, dt, tag=dram.name, name=dram.name + "_s")
                (eng or nc.sync).dma_start(t[:], dram.ap())
                return t

            w1_s = load(cpool, w1_d, [128, DH], bf16)
            dv128_s = load(cpool, dv128_d, [128, WPC])
            id_s = load(cpool, id_d, [128, 128], bf16)
            gi_s = load(cpool, gi_d, [128, 128], bf16)
            w2_s = load(cpool, w2_d, [DH, DOUT], bf16)
            dvrep_s = load(cpool, dvrep_d, [64, NPC], eng=nc.scalar)
            b1_s = load(cpool, b1_d, [DH, 1], eng=nc.scalar)
            b2_s = load(cpool, b2_d, [DOUT, 1], eng=nc.scalar)
            idaw_s = load(cpool, idaw_d, [DOUT, DOUT + 1], bf16,
                          eng=nc.scalar)
            ab_s = load(cpool, ab_d, [128, 1], eng=nc.scalar)
            ix2_s = load(cpool, ix2_d, [128, lp2["idx_cols"]],
                         mybir.dt.int16, eng=nc.scalar)
            dvl2_s = load(cpool, dvl2_d, [128, lp2["span_cols"]], bf16,
                          eng=nc.scalar)

            t2sb = cpool.tile([128, WPC * 64], bf16, tag="t2sb", name="t2sb")
            h2at = cpool.tile([128, WPC * 33], f32, tag="h2at", name="h2at")
            nc.vector.memset(t2sb[:], 0.0)

            with tc.tile_pool(name="dram", bufs=1, space="DRAM") as dpool:
                shard_t = {}
                full_t = {}
                loc_t = {}
                for bi, (w_lo, w_hi, slots, base) in enumerate(cfg.tchunks):
                    shard_t[bi] = dpool.tile(
                        [slots, DH], bf16, tag=f"sh2_{bi}", name=f"sh2_{bi}")
                    full_t[bi] = dpool.tile(
                        [NC * slots, DH], bf16, tag=f"fu2_{bi}",
                        name=f"fu2_{bi}", addr_space="Shared")
                    # all buckets read Shared-direct: measured faster than
                    # copying to local DRAM (copies contend with gather
                    # drain bandwidth)

                def distribute(bi):
                    w_lo, w_hi, slots, base = cfg.tchunks[bi]
                    sh, fu = shard_t[bi], full_t[bi]
                    nc.sync.dma_start(
                        sh[:].rearrange("(w p) f -> p w f", p=128),
                        t2sb[:, w_lo * 64:w_hi * 64].rearrange(
                            "p (w f) -> p w f", f=64))
                    nc.gpsimd.collective_compute(
                        "AllGather", mybir.AluOpType.bypass,
                        replica_groups=rg, ins=[sh[:]], outs=[fu[:]])
                    if bi in loc_t:
                        nc.sync.dma_start(loc_t[bi][:], fu[:])

                # ---- shared aggregation loop
                def aggregate(lp, fvs, ix_s, dvl_s, used, sl_lhs, flush_fn,
                              inject):
                    span_off, nspans = lp["span_off"], lp["nspans"]
                    max_nsp = lp["max_nsp"]
                    with (
                        tc.tile_pool(name="gp", bufs=cfg.GPBUFS) as gp,
                        tc.tile_pool(name="sp", bufs=2) as sp,
                        tc.tile_pool(name="aps", bufs=4, space="PSUM") as aps,
                        tc.tile_pool(name="fsb", bufs=3) as fsb,
                        tc.tile_pool(name="fps", bufs=2, space="PSUM") as fps,
                    ):
                        Ss = {}
                        ptiles = {}
                        for s in range(lp["nslot"] + 1):
                            gS = s - SHIFTC + 1
                            if 0 <= gS < G:
                                for h in range(lp["NH"]):
                                    nsp = nspans[(gS, h)]
                                    sc0 = span_off[(gS, h)]
                                    S = sp.tile([128, max_nsp * 128], bf16,
                                                tag=f"S{h}", name=f"S{h}")
                                    nc.vector.tensor_tensor(
                                        out=S[:, :nsp * 128].rearrange(
                                            "p (c j) -> p c j", j=128),
                                        in0=dvl_s[:, sc0:sc0 + nsp]
                                        .unsqueeze(2)
                                        .broadcast_to((128, nsp, 128)),
                                        in1=gi_s[:].unsqueeze(1)
                                        .broadcast_to((128, nsp, 128)),
                                        op=mybir.AluOpType.is_equal,
                                    )
                                    Ss[(gS, h)] = S
                            gM = s - SHIFTC
                            if 0 <= gM < G:
                                w0, nw = cfg.groups[gM]
                                for wl in range(nw):
                                    w = w0 + wl
                                    mlist = lp["wlist"][gM][wl]
                                    ps = aps.tile([used, 128], f32,
                                                  tag="agg", name="agg")
                                    nc.tensor.matmul(
                                        ps[:], lhsT=sl_lhs(w), rhs=id_s[:],
                                        start=True, stop=(len(mlist) == 0))
                                    for i, (h, pid, off, scol) in enumerate(
                                            mlist):
                                        pt = ptiles[pid]
                                        bb = off * 128 + h * 64
                                        sc = (scol - span_off[(gM, h)]) * 128
                                        nc.tensor.matmul(
                                            ps[:],
                                            lhsT=pt[:, bb:bb + used],
                                            rhs=Ss[(gM, h)][:, sc:sc + 128],
                                            start=False,
                                            stop=(i == len(mlist) - 1))
                                    flush_fn(w, ps, fsb, fps)
                            for pid in lp["slots"][s]:
                                (g, h, b, eoff, nn) = lp["pieces"][pid]
                                pt = gp.tile([128, GSPLIT], bf16,
                                             tag=f"gt{qctr[0] % NTAGS}",
                                             name=f"gt{qctr[0] % NTAGS}")
                                nc.gpsimd.dma_gather(
                                    out_ap=pt[:, 0:nn].rearrange(
                                        "p (c d) -> p c d", d=128),
                                    in_ap=fvs[b],
                                    idxs_ap=ix_s[:, eoff // 16:
                                                 (eoff + nn) // 16],
                                    num_idxs=nn, num_idxs_reg=nn,
                                    elem_size=128, elem_step=128,
                                    queue_num=qctr[0] % 4,
                                    single_packet=False,
                                )
                                qctr[0] += 1
                                ptiles[pid] = pt
                            inject(s)

                # ---- layer-1 flush: ps [128 xfeat, 128 slots] ->
                # zT bf16 -> W1 projection (feat-major) -> dinv,relu ->
                # t2 table values
                def flush1(w, ps, fsb, fps):
                    zT = fsb.tile([128, 128], bf16, tag="zT", name="zT")
                    nc.scalar.activation(zT[:], ps[:], func=AF.Copy)
                    hps = fps.tile([DH, 128], f32, tag="hps", name="hps")
                    nc.tensor.matmul(hps[:], lhsT=w1_s[:], rhs=zT[:],
                                     start=True, stop=True)
                    a = fsb.tile([64, 128], f32, tag="a", name="a")
                    nc.vector.tensor_tensor(
                        out=a[:], in0=hps[:],
                        in1=dvrep_s[:, w * 128:(w + 1) * 128],
                        op=mybir.AluOpType.mult)
                    hT = fsb.tile([64, 128], bf16, tag="hT", name="hT")
                    nc.scalar.activation(hT[:], a[:], func=AF.Relu,
                                         bias=b1_s[:, 0:1])
                    t2ps = fps.tile([128, DOUT], f32, tag="t2ps", name="t2ps")
                    nc.tensor.matmul(t2ps[:], lhsT=hT[:], rhs=w2_s[:],
                                     start=True, stop=True)
                    nc.scalar.activation(
                        t2sb[:, w * 64:w * 64 + DOUT], t2ps[:],
                        func=AF.Copy, scale=dv128_s[:, w:w + 1])

                def sl1(w):
                    return xo_s[:, w * 128:(w + 1) * 128]

                # t2 bucket bi: fire its distribute two slots after its
                # last group's matmuls; the final bucket fires from L2's
                # slot 0 (its readers are C pieces at slot >= 1).
                fire_at = {}
                for bi, (w_lo, w_hi, slots, base) in enumerate(
                        cfg.tchunks[:-1]):
                    gl = max(g for g, (gw0, gnw) in enumerate(cfg.groups)
                             if gw0 < w_hi)
                    fire_at.setdefault(
                        min(gl + SHIFTC + 1, lp1["nslot"]), []).append(bi)

                def inject1(s):
                    for bi in fire_at.get(s, []):
                        distribute(bi)

                fvs1 = [xnm_d.ap()[base:base + NC * slots, :]
                        for (w_lo, w_hi, slots, base) in cfg.tchunks]
                with tc.tile_pool(name="pl1", bufs=1) as pl1:
                    xo_s = load(pl1, xo_d, [128, NPC], bf16)
                    ix1_s = load(pl1, ix1_d, [128, lp1["idx_cols"]],
                                 mybir.dt.int16)
                    dvl1_s = load(pl1, dvl1_d, [128, lp1["span_cols"]],
                                  bf16)
                    aggregate(lp1, fvs1, ix1_s, dvl1_s, 128, sl1, flush1,
                              inject1)

                # ---- layer-2 flush
                def flush2(w, ps, fsb, fps):
                    a2 = fsb.tile([DOUT, 128], f32, tag="a2", name="a2")
                    nc.vector.tensor_tensor(
                        out=a2[:], in0=ps[:],
                        in1=dvrep_s[:DOUT, w * 128:(w + 1) * 128],
                        op=mybir.AluOpType.mult)
                    h2T = fsb.tile([DOUT, 128], bf16, tag="h2T", name="h2T")
                    nc.scalar.activation(h2T[:], a2[:], func=AF.Relu,
                                         bias=b2_s[:, 0:1])
                    gps = fps.tile([128, DOUT + 1], f32, tag="gps", name="gps")
                    nc.tensor.matmul(gps[:], lhsT=h2T[:], rhs=idaw_s[:],
                                     start=True, stop=True)
                    nc.scalar.activation(
                        h2at[:, w * 33:(w + 1) * 33], gps[:], func=AF.Copy)

                def sl2(w):
                    return t2sb[:, w * 64:w * 64 + DOUT]

                def inject2(s):
                    if s == 0:
                        distribute(cfg.NB - 1)

                fvs2 = [(loc_t[bi] if bi in loc_t else full_t[bi])[:]
                        .rearrange("(a b) d -> a (b d)", b=2)
                        for bi in range(NB)]
                aggregate(lp2, fvs2, ix2_s, dvl2_s, DOUT, sl2, flush2,
                          inject2)

                # ---- attention gate tail
                with tc.tile_pool(name="tail", bufs=1) as tp:
                    atall = tp.tile([128, WPC], f32, tag="atall", name="atall")
                    nc.scalar.activation(
                        atall[:],
                        h2at[:].rearrange("p (w q) -> p w q", q=33)[:, :, 32],
                        func=AF.Sigmoid, bias=ab_s[:, 0:1])
                    oall = tp.tile([128, WPC * DOUT], f32, tag="oall",
                                   name="oall")
                    nc.vector.tensor_tensor(
                        out=oall[:].rearrange("p (w f) -> p w f", f=DOUT),
                        in0=h2at[:].rearrange(
                            "p (w q) -> p w q", q=33)[:, :, 0:DOUT],
                        in1=atall[:].unsqueeze(2)
                        .broadcast_to((128, WPC, DOUT)),
                        op=mybir.AluOpType.mult)
                    nc.sync.dma_start(
                        out_d.ap().rearrange("(w p) f -> p w f", p=128),
                        oall[:].rearrange("p (w f) -> p w f", f=DOUT))

    nc.compile()
    return nc


# ---------------------------------------------------------------------------
# entry point
# ---------------------------------------------------------------------------


def _make_in_maps(cfg, host, W1, b1, W2, b2, attn_w, attn_b):
    import ml_dtypes
    bf16 = ml_dtypes.bfloat16
    giota = np.tile(np.arange(2, 130, dtype=np.float32),
                    (128, 1)).astype(bf16)
    idaw = np.concatenate(
        [np.eye(cfg.DOUT, dtype=np.float32),
         np.asarray(attn_w, np.float32).reshape(cfg.DOUT, 1)],
        axis=1).astype(bf16)
    in_maps = []
    for c in range(cfg.NC):
        in_maps.append({
            "xnm": host["xnm"],
            "xown": host["xown"][c],
            "w1": np.asarray(W1, np.float32).astype(bf16),
            "w2": np.asarray(W2, np.float32).astype(bf16),
            "dv128": host["dv128"][c],
            "dvrep": host["dvrep"][c],
            "ident": np.eye(128, dtype=np.float32).astype(bf16),
            "b1c": np.asarray(b1, np.float32).reshape(cfg.DH, 1),
            "b2c": np.asarray(b2, np.float32).reshape(cfg.DOUT, 1),
            "idaw": idaw,
            "abc": np.full((128, 1),
                           np.asarray(attn_b, np.float32).reshape(-1)[0],
                           np.float32),
            "gi128": giota,
            "ix1": host["ix1"][c],
            "dvl1": host["dvl1"][c],
            "ix2": host["ix2"][c],
            "dvl2": host["dvl2"][c],
        })
    return in_maps


def run(x, edge_index, W1, b1, W2, b2, attn_w, attn_b, cfg=None,
        backend="hw", trace=False):
    cfg = cfg or FULL
    plan, host = prep(x, edge_index, cfg)
    nc = build(cfg, plan)
    in_maps = _make_in_maps(cfg, host, W1, b1, W2, b2, attn_w, attn_b)

    if backend == "sim":
        from concourse.bass_interp import MultiCoreSim
        sim = MultiCoreSim(nc, num_cores=cfg.NC, trace=False)
        for c, core in enumerate(sim.cores.values()):
            for name, arr in in_maps[c].items():
                core.tensor(name)[:] = arr
        sim.simulate()
        outs = [core.tensor("out_sh").copy() for core in sim.cores.values()]
        exec_ns = None
    else:
        from concourse import bass_utils
        from concourse.bass_interp import get_hw_module
        old = nc.m
        nc.m = get_hw_module(nc.m)
        try:
            res = bass_utils.run_bass_kernel_spmd(
                nc, in_maps, core_ids=list(range(cfg.NC)), trace=trace)
        finally:
            nc.m = old
        outs = [res.results[c]["out_sh"] for c in range(cfg.NC)]
        exec_ns = res.exec_time_ns

    full = np.concatenate(outs, axis=0)   # [TOT, DOUT] in slot order
    out = full[host["pos"]]               # unpermute -> [N, DOUT]
    return np.ascontiguousarray(out), exec_ns


def kernel(x, edge_index, W1, b1, W2, b2, attn_w, attn_b):
    out, _ = run(x, edge_index, W1, b1, W2, b2, attn_w, attn_b,
                 cfg=FULL, backend="hw", trace=False)
    return out


# revision 31
# speedup vs baseline: 1.0327x; 1.0162x over previous
"""Trainium2 Bass kernel v6 for a 2-layer GCN with data-aware attention gate.

Math (per reference):
    src,dst = edges + self-loops; deg = bincount(dst); dinv = rsqrt(deg)
    norm = dinv[src]*dinv[dst]
    h1 = relu(segsum(norm * (x@W1)[src], dst) + b1)
    h2 = relu(segsum(norm * (h1@W2)[src], dst) + b2)
    out = h2 * sigmoid(h2@attn_w + attn_b)

v6 design (8 NeuronCores, dst-sharded, 128-slot windows):
  - Layer 1 exploits linearity: segsum(norm*(xW1)[src]) =
    (segsum(dinv_src*x[src]) * dinv_dst) @ W1.  The gather table is a
    HOST-prescaled (dinv*x) node-major bf16 tensor in table-row order --
    an ExternalInput, so layer 1 needs no device-side table build, no
    collective, and no copies.  Elements are single nodes (256B), indices
    are bucket-relative rows (int16-safe), no parity split.  W1 is
    applied per window right after aggregation (one extra matmul).
  - Layer 2 tables (dinv*(h1@W2), 64B rows zero-padded to 128B) are
    pair-packed 256B elements with a parity split, distributed in 3
    window-chunk buckets via AllGather+copy fired as soon as each
    chunk's flushes complete, overlapping layer-1 aggregation.
  - Near-exact gather streams per (group[, parity], bucket): per-window
    segments sized max-over-cores at element granularity (SPMD); chunks
    straddling window boundaries get one extra one-hot column per extra
    window (other windows' edges masked to -1).
  - Pieces are issued diagonally (A at slot g; C,B at slot g+1, issued
    before A of g+1) with matmuls for group g at slot g+2, so table
    buckets pipeline behind gather descriptor generation and the live
    gather-buffer window stays below NTAGS*GPBUFS.
  - Gathers round-robin the 4 SWDGE queues (each queue's descriptor
    generation runs on its own Q7 core pair).
  - Transposed aggregation: matmul(lhsT=gathered-chunk[128e x used],
    rhs=S[128e x 128 slots]) accumulates feat-major [used, 128] PSUM per
    window; self-loops via one identity matmul per window.
"""

import sys

import numpy as np

_CONC = "/opt/trn_rl_repo"
if _CONC not in sys.path:
    sys.path.insert(0, _CONC)

# ---------------------------------------------------------------------------
# configuration
# ---------------------------------------------------------------------------


class Cfg:
    def __init__(self, N=50000, DIN=128, DH=64, DOUT=32, NC=8, WPC=49, WPG=5,
                 GSPLIT=1024, NTAGS=12, GPBUFS=2, SHIFTB=1, SHIFTC=2):
        self.N, self.DIN, self.DH, self.DOUT = N, DIN, DH, DOUT
        self.NC, self.WPC, self.WPG = NC, WPC, WPG
        self.GSPLIT, self.NTAGS, self.GPBUFS = GSPLIT, NTAGS, GPBUFS
        self.SHIFTB, self.SHIFTC = SHIFTB, SHIFTC
        assert GSPLIT % 128 == 0
        self.groups = []
        w0 = 0
        while w0 < WPC:
            nw = min(WPG, WPC - w0)
            self.groups.append((w0, nw))
            w0 += nw
        self.G = len(self.groups)
        self.NPC = WPC * 128            # slots per core
        self.TOT = NC * self.NPC
        assert self.N <= self.TOT
        # table distribution buckets (window ranges; even slot counts)
        self.NB = 3
        self.tchunks = []               # (w_lo, w_hi, slots, table_base)
        base = 0
        for (w_lo, w_hi) in [(0, 20), (20, 40), (40, WPC)]:
            slots = (w_hi - w_lo) * 128
            self.tchunks.append((w_lo, w_hi, slots, base))
            base += NC * slots
        assert base == self.TOT


FULL = Cfg()

# ---------------------------------------------------------------------------
# host-side graph prep (structure only)
# ---------------------------------------------------------------------------


def _assign_slots(load, cfg):
    """LPT-deal nodes into NC*WPC bins of <=128 slots, balancing `load`."""
    import heapq

    nbins = cfg.NC * cfg.WPC
    order = np.argsort(-load, kind="stable")
    heap = [(0, b) for b in range(nbins)]
    heapq.heapify(heap)
    count = np.zeros(nbins, np.int64)
    pos = np.empty(cfg.N, np.int64)
    for n in order:
        l, b = heapq.heappop(heap)
        pos[n] = b * 128 + count[b]
        count[b] += 1
        if count[b] < 128:
            heapq.heappush(heap, (l + int(load[n]), b))
    return pos


def _build_streams(cfg, c_e, w_e, b_e, idx_e, h_e, dval_e, NH):
    """Shared stream/span/schedule builder for one layer.

    idx_e: bucket-relative gather element index per edge (int16-safe).
    h_e: parity half per edge (all zero when NH == 1).
    Returns (layerplan, ix_all, dvl_all).
    """
    NC, WPC, G, NB, GSPLIT = cfg.NC, cfg.WPC, cfg.G, cfg.NB, cfg.GSPLIT
    PC = GSPLIT // 128

    key = (((c_e * NH + h_e) * NB + b_e) * WPC + w_e) * 32768 + idx_e
    order_e = np.argsort(key, kind="stable")
    se_g = idx_e[order_e]
    se_d = dval_e[order_e]
    bkey = ((c_e * NH + h_e) * NB + b_e) * WPC + w_e
    se_key = bkey[order_e]
    bounds = np.searchsorted(se_key, np.arange(NC * NH * NB * WPC + 1))

    def bin_of(c, h, b, w):
        return ((c * NH + h) * NB + b) * WPC + w

    cnt = (bounds[1:] - bounds[:-1]).reshape(NC, NH, NB, WPC)
    segL = cnt.max(axis=0)                          # [NH, NB, WPC]

    elem_off = {}
    span_off = {}
    nspans = {}
    stream_L = {}
    seg_start = {}
    spans = {}
    pieces = []
    pieces_by_gb = {}
    wlist = [[[] for _ in range(nw)] for (w0, nw) in cfg.groups]
    etot = 0
    stot = 0
    for g, (w0, nw) in enumerate(cfg.groups):
        for h in range(NH):
            span_off[(g, h)] = stot
            for b in range(NB):
                starts = np.zeros(nw + 1, np.int64)
                for wl in range(nw):
                    starts[wl + 1] = starts[wl] + segL[h, b, w0 + wl]
                L = int(np.ceil(max(int(starts[nw]), 1) / 128) * 128)
                elem_off[(g, h, b)] = etot
                seg_start[(g, h, b)] = starts
                stream_L[(g, h, b)] = L
                plist = []
                o0 = 0
                while o0 < L:
                    nn = min(GSPLIT, L - o0)
                    pid = len(pieces)
                    pieces.append((g, h, b, etot + o0, nn))
                    plist.append(pid)
                    o0 += nn
                pieces_by_gb.setdefault((g, b), []).extend(plist)
                nch = L // 128
                sp_list = []
                for k2 in range(nch):
                    a2, b2 = k2 * 128, k2 * 128 + 128
                    wls = [wl for wl in range(nw)
                           if starts[wl] < b2 and starts[wl + 1] > a2]
                    if not wls:
                        wls = [nw - 1]
                    sp_list.append(wls)
                    for wl in wls:
                        pj = k2 // PC
                        off = k2 - pj * PC
                        wlist[g][wl].append((h, plist[pj], off, stot))
                        stot += 1
                spans[(g, h, b)] = sp_list
                etot += L
            nspans[(g, h)] = stot - span_off[(g, h)]
    idx_cols = etot // 16
    span_cols = stot
    max_nsp = max(nspans.values())

    NSLOT = G + cfg.SHIFTC - 1
    slots_sched = []
    for s in range(NSLOT + 1):
        ids = []
        for b, shift in ((2, cfg.SHIFTB), (1, cfg.SHIFTB), (0, 0)):
            g = s - shift
            if 0 <= g < G:
                ids.extend(pieces_by_gb.get((g, b), []))
        slots_sched.append(ids)

    ix_all = np.zeros((NC, 128, idx_cols), np.int16)
    dvl_all = np.full((NC, 128, span_cols), -1.0, np.float32)
    for c in range(NC):
        gi_full = np.zeros(etot, np.int64)
        dv_full = np.full(etot, -1.0, np.float32)
        wv_full = np.full(etot, -1, np.int64)
        for g, (w0, nw) in enumerate(cfg.groups):
            for h in range(NH):
                for b in range(NB):
                    base = elem_off[(g, h, b)]
                    starts = seg_start[(g, h, b)]
                    for wl in range(nw):
                        k = bin_of(c, h, b, w0 + wl)
                        lo, hi = bounds[k], bounds[k + 1]
                        n = hi - lo
                        o = base + starts[wl]
                        gi_full[o:o + n] = se_g[lo:hi]
                        dv_full[o:o + n] = se_d[lo:hi]
                        wv_full[o:o + n] = wl
        ix_all[c] = np.tile(
            gi_full.reshape(idx_cols, 16).T.astype(np.int16), (8, 1))
        scol = 0
        for g, (w0, nw) in enumerate(cfg.groups):
            for h in range(NH):
                for b in range(NB):
                    base = elem_off[(g, h, b)]
                    for k2, wls in enumerate(spans[(g, h, b)]):
                        sl = slice(base + k2 * 128, base + k2 * 128 + 128)
                        for wl in wls:
                            dvl_all[c, :, scol] = np.where(
                                wv_full[sl] == wl, dv_full[sl], -1.0)
                            scol += 1
        assert scol == span_cols

    lp = dict(idx_cols=idx_cols, span_cols=span_cols, max_nsp=max_nsp,
              span_off=span_off, nspans=nspans, pieces=pieces,
              slots=slots_sched, wlist=wlist, etot=etot, nslot=NSLOT,
              NH=NH)
    return lp, ix_all, dvl_all


def prep(x, edge_index, cfg):
    N, NC, WPC = cfg.N, cfg.NC, cfg.WPC
    NPC, DIN = cfg.NPC, cfg.DIN

    src = edge_index[0].astype(np.int64)
    dst = edge_index[1].astype(np.int64)
    deg_in = np.bincount(dst, minlength=N).astype(np.int64)
    deg = (deg_in + 1).astype(np.float32)          # + self-loop
    dinv = (1.0 / np.sqrt(np.maximum(deg, 1e-12))).astype(np.float32)

    pos = _assign_slots(deg_in, cfg)

    dinv_slot = np.ones(cfg.TOT, np.float32)
    dinv_slot[pos] = dinv

    # slot (core-major: c*NPC + s) -> (bucket, bucket-relative row)
    trow_b = np.empty(cfg.TOT, np.int64)
    trow_r = np.empty(cfg.TOT, np.int64)
    s_all = np.arange(NPC)
    for bi, (w_lo, w_hi, slots, base) in enumerate(cfg.tchunks):
        sel = s_all[(s_all >= w_lo * 128) & (s_all < w_hi * 128)]
        for c in range(NC):
            trow_b[c * NPC + sel] = bi
            trow_r[c * NPC + sel] = c * slots + (sel - w_lo * 128)

    s_pos = pos[src]
    d_pos = pos[dst]
    b_e = trow_b[s_pos]
    r_e = trow_r[s_pos]
    c_e = d_pos // NPC
    w_e = (d_pos % NPC) // 128
    dval_e = (d_pos % 128 + 2).astype(np.float32)

    # layer 1: single-node 256B elements, bucket-relative row index
    assert r_e.max() < 32768
    lp1, ix1, dvl1 = _build_streams(
        cfg, c_e, w_e, b_e, r_e, np.zeros_like(r_e), dval_e, NH=1)
    # layer 2: pair-packed 256B elements, parity split
    lp2, ix2, dvl2 = _build_streams(
        cfg, c_e, w_e, b_e, r_e >> 1, r_e & 1, dval_e, NH=2)

    import ml_dtypes
    bf16 = ml_dtypes.bfloat16

    # host-prescaled gather table for layer 1: (dinv*x) node-major,
    # table-row order; plus the own-shard copy for self-loop matmuls.
    X_all = np.zeros((cfg.TOT, DIN), np.float32)
    X_all[pos] = np.asarray(x, np.float32)
    Xs = X_all * dinv_slot[:, None]
    xnm = np.zeros((cfg.TOT, DIN), bf16)
    for bi, (w_lo, w_hi, slots, base) in enumerate(cfg.tchunks):
        sel = (trow_b == bi)
        rows = np.zeros((NC * slots, DIN), bf16)
        rows[trow_r[sel]] = Xs[sel].astype(bf16)
        xnm[base:base + NC * slots] = rows
    xown = np.zeros((NC, 128, NPC), bf16)
    for c in range(NC):
        sl = Xs[c * NPC:(c + 1) * NPC].astype(bf16)   # [NPC, 128]
        xown[c] = sl.reshape(WPC, 128, DIN).transpose(1, 0, 2).reshape(
            128, NPC)

    dv128 = np.zeros((NC, 128, WPC), np.float32)
    dvrep = np.zeros((NC, 64, NPC), np.float32)
    for c in range(NC):
        ds = dinv_slot[c * NPC:(c + 1) * NPC]
        dv128[c] = ds.reshape(WPC, 128).T
        dvrep[c] = np.tile(ds, (64, 1))

    plan = dict(lp1=lp1, lp2=lp2)
    host = dict(xnm=xnm, xown=xown, dv128=dv128, dvrep=dvrep,
                ix1=ix1, dvl1=dvl1.astype(bf16),
                ix2=ix2, dvl2=dvl2.astype(bf16),
                pos=pos, trow_b=trow_b, trow_r=trow_r)
    return plan, host


# ---------------------------------------------------------------------------
# device kernel
# ---------------------------------------------------------------------------


def build(cfg, plan):
    import concourse.bass as bass  # noqa: F401
    import concourse.mybir as mybir
    import concourse.tile as tile
    from concourse import bacc

    NC, WPC, G, NB = cfg.NC, cfg.WPC, cfg.G, cfg.NB
    NPC = cfg.NPC
    DH, DOUT = cfg.DH, cfg.DOUT
    GSPLIT, NTAGS = cfg.GSPLIT, cfg.NTAGS
    f32 = mybir.dt.float32
    bf16 = mybir.dt.bfloat16
    AF = mybir.ActivationFunctionType
    SHIFTC = cfg.SHIFTC
    lp1, lp2 = plan["lp1"], plan["lp2"]

    nc = bacc.Bacc(
        "TRN2", target_bir_lowering=False, debug=False,
        num_devices=NC, num_swdge_queues=4,
    )

    xnm_d = nc.dram_tensor("xnm", [cfg.TOT, 128], bf16,
                           kind="ExternalInput")
    xo_d = nc.dram_tensor("xown", [128, NPC], bf16, kind="ExternalInput")
    w1_d = nc.dram_tensor("w1", [128, DH], bf16, kind="ExternalInput")
    w2_d = nc.dram_tensor("w2", [DH, DOUT], bf16, kind="ExternalInput")
    dv128_d = nc.dram_tensor("dv128", [128, WPC], f32, kind="ExternalInput")
    dvrep_d = nc.dram_tensor("dvrep", [64, NPC], f32, kind="ExternalInput")
    id_d = nc.dram_tensor("ident", [128, 128], bf16, kind="ExternalInput")
    b1_d = nc.dram_tensor("b1c", [DH, 1], f32, kind="ExternalInput")
    b2_d = nc.dram_tensor("b2c", [DOUT, 1], f32, kind="ExternalInput")
    idaw_d = nc.dram_tensor("idaw", [DOUT, DOUT + 1], bf16,
                            kind="ExternalInput")
    ab_d = nc.dram_tensor("abc", [128, 1], f32, kind="ExternalInput")
    gi_d = nc.dram_tensor("gi128", [128, 128], bf16, kind="ExternalInput")
    ix1_d = nc.dram_tensor("ix1", [128, lp1["idx_cols"]], mybir.dt.int16,
                           kind="ExternalInput")
    dvl1_d = nc.dram_tensor("dvl1", [128, lp1["span_cols"]], bf16,
                            kind="ExternalInput")
    ix2_d = nc.dram_tensor("ix2", [128, lp2["idx_cols"]], mybir.dt.int16,
                           kind="ExternalInput")
    dvl2_d = nc.dram_tensor("dvl2", [128, lp2["span_cols"]], bf16,
                            kind="ExternalInput")
    out_d = nc.dram_tensor("out_sh", [NPC, DOUT], f32, kind="ExternalOutput")

    rg = [list(range(NC))]
    qctr = [0]

    with tile.TileContext(nc) as tc:
        with tc.tile_pool(name="const", bufs=1) as cpool:
            def load(pool, dram, shape, dt=f32, eng=None):
                t = pool.tile(shape, dt, tag=dram.name, name=dram.name + "_s")
                (eng or nc.sync).dma_start(t[:], dram.ap())
                return t

            w1_s = load(cpool, w1_d, [128, DH], bf16)
            dv128_s = load(cpool, dv128_d, [128, WPC])
            id_s = load(cpool, id_d, [128, 128], bf16)
            gi_s = load(cpool, gi_d, [128, 128], bf16)
            w2_s = load(cpool, w2_d, [DH, DOUT], bf16)
            dvrep_s = load(cpool, dvrep_d, [64, NPC], eng=nc.scalar)
            b1_s = load(cpool, b1_d, [DH, 1], eng=nc.scalar)
            b2_s = load(cpool, b2_d, [DOUT, 1], eng=nc.scalar)
            idaw_s = load(cpool, idaw_d, [DOUT, DOUT + 1], bf16,
                          eng=nc.scalar)
            ab_s = load(cpool, ab_d, [128, 1], eng=nc.scalar)
            ix2_s = load(cpool, ix2_d, [128, lp2["idx_cols"]],
                         mybir.dt.int16, eng=nc.scalar)
            dvl2_s = load(cpool, dvl2_d, [128, lp2["span_cols"]], bf16,
                          eng=nc.scalar)

            t2sb = cpool.tile([128, WPC * 64], bf16, tag="t2sb", name="t2sb")
            h2at = cpool.tile([128, WPC * 33], f32, tag="h2at", name="h2at")
            nc.vector.memset(t2sb[:], 0.0)

            with tc.tile_pool(name="dram", bufs=1, space="DRAM") as dpool:
                shard_t = {}
                full_t = {}
                loc_t = {}
                for bi, (w_lo, w_hi, slots, base) in enumerate(cfg.tchunks):
                    shard_t[bi] = dpool.tile(
                        [slots, DH], bf16, tag=f"sh2_{bi}", name=f"sh2_{bi}")
                    full_t[bi] = dpool.tile(
                        [NC * slots, DH], bf16, tag=f"fu2_{bi}",
                        name=f"fu2_{bi}", addr_space="Shared")
                    # all buckets read Shared-direct: measured faster than
                    # copying to local DRAM (copies contend with gather
                    # drain bandwidth)

                def distribute(bi):
                    w_lo, w_hi, slots, base = cfg.tchunks[bi]
                    sh, fu = shard_t[bi], full_t[bi]
                    nc.sync.dma_start(
                        sh[:].rearrange("(w p) f -> p w f", p=128),
                        t2sb[:, w_lo * 64:w_hi * 64].rearrange(
                            "p (w f) -> p w f", f=64))
                    nc.gpsimd.collective_compute(
                        "AllGather", mybir.AluOpType.bypass,
                        replica_groups=rg, ins=[sh[:]], outs=[fu[:]])
                    if bi in loc_t:
                        nc.sync.dma_start(loc_t[bi][:], fu[:])

                # ---- shared aggregation loop
                def aggregate(lp, fvs, ix_s, dvl_s, used, sl_lhs, flush_fn,
                              inject):
                    span_off, nspans = lp["span_off"], lp["nspans"]
                    max_nsp = lp["max_nsp"]
                    with (
                        tc.tile_pool(name="gp", bufs=cfg.GPBUFS) as gp,
                        tc.tile_pool(name="sp", bufs=2) as sp,
                        tc.tile_pool(name="aps", bufs=4, space="PSUM") as aps,
                        tc.tile_pool(name="fsb", bufs=3) as fsb,
                        tc.tile_pool(name="fps", bufs=2, space="PSUM") as fps,
                    ):
                        Ss = {}
                        ptiles = {}
                        for s in range(lp["nslot"] + 1):
                            gS = s - SHIFTC + 1
                            if 0 <= gS < G:
                                for h in range(lp["NH"]):
                                    nsp = nspans[(gS, h)]
                                    sc0 = span_off[(gS, h)]
                                    S = sp.tile([128, max_nsp * 128], bf16,
                                                tag=f"S{h}", name=f"S{h}")
                                    nc.vector.tensor_tensor(
                                        out=S[:, :nsp * 128].rearrange(
                                            "p (c j) -> p c j", j=128),
                                        in0=dvl_s[:, sc0:sc0 + nsp]
                                        .unsqueeze(2)
                                        .broadcast_to((128, nsp, 128)),
                                        in1=gi_s[:].unsqueeze(1)
                                        .broadcast_to((128, nsp, 128)),
                                        op=mybir.AluOpType.is_equal,
                                    )
                                    Ss[(gS, h)] = S
                            gM = s - SHIFTC
                            if 0 <= gM < G:
                                w0, nw = cfg.groups[gM]
                                for wl in range(nw):
                                    w = w0 + wl
                                    mlist = lp["wlist"][gM][wl]
                                    ps = aps.tile([used, 128], f32,
                                                  tag="agg", name="agg")
                                    nc.tensor.matmul(
                                        ps[:], lhsT=sl_lhs(w), rhs=id_s[:],
                                        start=True, stop=(len(mlist) == 0))
                                    for i, (h, pid, off, scol) in enumerate(
                                            mlist):
                                        pt = ptiles[pid]
                                        bb = off * 128 + h * 64
                                        sc = (scol - span_off[(gM, h)]) * 128
                                        nc.tensor.matmul(
                                            ps[:],
                                            lhsT=pt[:, bb:bb + used],
                                            rhs=Ss[(gM, h)][:, sc:sc + 128],
                                            start=False,
                                            stop=(i == len(mlist) - 1))
                                    flush_fn(w, ps, fsb, fps)
                            for pid in lp["slots"][s]:
                                (g, h, b, eoff, nn) = lp["pieces"][pid]
                                pt = gp.tile([128, GSPLIT], bf16,
                                             tag=f"gt{qctr[0] % NTAGS}",
                                             name=f"gt{qctr[0] % NTAGS}")
                                nc.gpsimd.dma_gather(
                                    out_ap=pt[:, 0:nn].rearrange(
                                        "p (c d) -> p c d", d=128),
                                    in_ap=fvs[b],
                                    idxs_ap=ix_s[:, eoff // 16:
                                                 (eoff + nn) // 16],
                                    num_idxs=nn, num_idxs_reg=nn,
                                    elem_size=128, elem_step=128,
                                    queue_num=qctr[0] % 4,
                                    single_packet=True,
                                )
                                qctr[0] += 1
                                ptiles[pid] = pt
                            inject(s)

                # ---- layer-1 flush: ps [128 xfeat, 128 slots] ->
                # zT bf16 -> W1 projection (feat-major) -> dinv,relu ->
                # t2 table values
                def flush1(w, ps, fsb, fps):
                    zT = fsb.tile([128, 128], bf16, tag="zT", name="zT")
                    nc.scalar.activation(zT[:], ps[:], func=AF.Copy)
                    hps = fps.tile([DH, 128], f32, tag="hps", name="hps")
                    nc.tensor.matmul(hps[:], lhsT=w1_s[:], rhs=zT[:],
                                     start=True, stop=True)
                    a = fsb.tile([64, 128], f32, tag="a", name="a")
                    nc.vector.tensor_tensor(
                        out=a[:], in0=hps[:],
                        in1=dvrep_s[:, w * 128:(w + 1) * 128],
                        op=mybir.AluOpType.mult)
                    hT = fsb.tile([64, 128], bf16, tag="hT", name="hT")
                    nc.scalar.activation(hT[:], a[:], func=AF.Relu,
                                         bias=b1_s[:, 0:1])
                    t2ps = fps.tile([128, DOUT], f32, tag="t2ps", name="t2ps")
                    nc.tensor.matmul(t2ps[:], lhsT=hT[:], rhs=w2_s[:],
                                     start=True, stop=True)
                    nc.scalar.activation(
                        t2sb[:, w * 64:w * 64 + DOUT], t2ps[:],
                        func=AF.Copy, scale=dv128_s[:, w:w + 1])

                def sl1(w):
                    return xo_s[:, w * 128:(w + 1) * 128]

                # t2 bucket bi: fire its distribute two slots after its
                # last group's matmuls; the final bucket fires from L2's
                # slot 0 (its readers are C pieces at slot >= 1).
                fire_at = {}
                for bi, (w_lo, w_hi, slots, base) in enumerate(
                        cfg.tchunks[:-1]):
                    gl = max(g for g, (gw0, gnw) in enumerate(cfg.groups)
                             if gw0 < w_hi)
                    fire_at.setdefault(
                        min(gl + SHIFTC + 1, lp1["nslot"]), []).append(bi)

                def inject1(s):
                    for bi in fire_at.get(s, []):
                        distribute(bi)

                fvs1 = [xnm_d.ap()[base:base + NC * slots, :]
                        for (w_lo, w_hi, slots, base) in cfg.tchunks]
                with tc.tile_pool(name="pl1", bufs=1) as pl1:
                    xo_s = load(pl1, xo_d, [128, NPC], bf16)
                    ix1_s = load(pl1, ix1_d, [128, lp1["idx_cols"]],
                                 mybir.dt.int16)
                    dvl1_s = load(pl1, dvl1_d, [128, lp1["span_cols"]],
                                  bf16)
                    aggregate(lp1, fvs1, ix1_s, dvl1_s, 128, sl1, flush1,
                              inject1)

                # ---- layer-2 flush
                def flush2(w, ps, fsb, fps):
                    a2 = fsb.tile([DOUT, 128], f32, tag="a2", name="a2")
                    nc.vector.tensor_tensor(
                        out=a2[:], in0=ps[:],
                        in1=dvrep_s[:DOUT, w * 128:(w + 1) * 128],
                        op=mybir.AluOpType.mult)
                    h2T = fsb.tile([DOUT, 128], bf16, tag="h2T", name="h2T")
                    nc.scalar.activation(h2T[:], a2[:], func=AF.Relu,
                                         bias=b2_s[:, 0:1])
                    gps = fps.tile([128, DOUT + 1], f32, tag="gps", name="gps")
                    nc.tensor.matmul(gps[:], lhsT=h2T[:], rhs=idaw_s[:],
                                     start=True, stop=True)
                    nc.scalar.activation(
                        h2at[:, w * 33:(w + 1) * 33], gps[:], func=AF.Copy)

                def sl2(w):
                    return t2sb[:, w * 64:w * 64 + DOUT]

                def inject2(s):
                    if s == 0:
                        distribute(cfg.NB - 1)

                fvs2 = [(loc_t[bi] if bi in loc_t else full_t[bi])[:]
                        .rearrange("(a b) d -> a (b d)", b=2)
                        for bi in range(NB)]
                aggregate(lp2, fvs2, ix2_s, dvl2_s, DOUT, sl2, flush2,
                          inject2)

                # ---- attention gate tail
                with tc.tile_pool(name="tail", bufs=1) as tp:
                    atall = tp.tile([128, WPC], f32, tag="atall", name="atall")
                    nc.scalar.activation(
                        atall[:],
                        h2at[:].rearrange("p (w q) -> p w q", q=33)[:, :, 32],
                        func=AF.Sigmoid, bias=ab_s[:, 0:1])
                    oall = tp.tile([128, WPC * DOUT], f32, tag="oall",
                                   name="oall")
                    nc.vector.tensor_tensor(
                        out=oall[:].rearrange("p (w f) -> p w f", f=DOUT),
                        in0=h2at[:].rearrange(
                            "p (w q) -> p w q", q=33)[:, :, 0:DOUT],
                        in1=atall[:].unsqueeze(2)
                        .broadcast_to((128, WPC, DOUT)),
                        op=mybir.AluOpType.mult)
                    nc.sync.dma_start(
                        out_d.ap().rearrange("(w p) f -> p w f", p=128),
                        oall[:].rearrange("p (w f) -> p w f", f=DOUT))

    nc.compile()
    return nc


# ---------------------------------------------------------------------------
# entry point
# ---------------------------------------------------------------------------


def _make_in_maps(cfg, host, W1, b1, W2, b2, attn_w, attn_b):
    import ml_dtypes
    bf16 = ml_dtypes.bfloat16
    giota = np.tile(np.arange(2, 130, dtype=np.float32),
                    (128, 1)).astype(bf16)
    idaw = np.concatenate(
        [np.eye(cfg.DOUT, dtype=np.float32),
         np.asarray(attn_w, np.float32).reshape(cfg.DOUT, 1)],
        axis=1).astype(bf16)
    in_maps = []
    for c in range(cfg.NC):
        in_maps.append({
            "xnm": host["xnm"],
            "xown": host["xown"][c],
            "w1": np.asarray(W1, np.float32).astype(bf16),
            "w2": np.asarray(W2, np.float32).astype(bf16),
            "dv128": host["dv128"][c],
            "dvrep": host["dvrep"][c],
            "ident": np.eye(128, dtype=np.float32).astype(bf16),
            "b1c": np.asarray(b1, np.float32).reshape(cfg.DH, 1),
            "b2c": np.asarray(b2, np.float32).reshape(cfg.DOUT, 1),
            "idaw": idaw,
            "abc": np.full((128, 1),
                           np.asarray(attn_b, np.float32).reshape(-1)[0],
                           np.float32),
            "gi128": giota,
            "ix1": host["ix1"][c],
            "dvl1": host["dvl1"][c],
            "ix2": host["ix2"][c],
            "dvl2": host["dvl2"][c],
        })
    return in_maps


def run(x, edge_index, W1, b1, W2, b2, attn_w, attn_b, cfg=None,
        backend="hw", trace=False):
    cfg = cfg or FULL
    plan, host = prep(x, edge_index, cfg)
    nc = build(cfg, plan)
    in_maps = _make_in_maps(cfg, host, W1, b1, W2, b2, attn_w, attn_b)

    if backend == "sim":
        from concourse.bass_interp import MultiCoreSim
        sim = MultiCoreSim(nc, num_cores=cfg.NC, trace=False)
        for c, core in enumerate(sim.cores.values()):
            for name, arr in in_maps[c].items():
                core.tensor(name)[:] = arr
        sim.simulate()
        outs = [core.tensor("out_sh").copy() for core in sim.cores.values()]
        exec_ns = None
    else:
        from concourse import bass_utils
        from concourse.bass_interp import get_hw_module
        old = nc.m
        nc.m = get_hw_module(nc.m)
        try:
            res = bass_utils.run_bass_kernel_spmd(
                nc, in_maps, core_ids=list(range(cfg.NC)), trace=trace)
        finally:
            nc.m = old
        outs = [res.results[c]["out_sh"] for c in range(cfg.NC)]
        exec_ns = res.exec_time_ns

    full = np.concatenate(outs, axis=0)   # [TOT, DOUT] in slot order
    out = full[host["pos"]]               # unpermute -> [N, DOUT]
    return np.ascontiguousarray(out), exec_ns


def kernel(x, edge_index, W1, b1, W2, b2, attn_w, attn_b):
    out, _ = run(x, edge_index, W1, b1, W2, b2, attn_w, attn_b,
                 cfg=FULL, backend="hw", trace=False)
    return out
